# revision 17
# baseline (speedup 1.0000x reference)
"""Trainium2 Bass kernel for the CLF-QP network.

Math (per sample, fp32):
    a1 = tanh(W1 x + b1); a2 = tanh(W2 a1 + b2); V = 0.5||a2||^2
    grad_V = W1^T ((1-a1^2) . (W2^T ((1-a2^2) . a2)))
    f = A_f x; L_f = <grad_V, f>; L_g = Gmat^T grad_V; u_nom = -K x
    s = L_f + V + <L_g, u_nom>; r = relu(s) / (1 + 100||L_g||^2)
    u = u_nom - 100 r L_g; Vdot = L_f + <L_g, u>

Strategy: pure data parallel over 8 NeuronCores (4096 rows each).
On-chip layout is feature-major ([feature, batch]) so weights are the
stationary matmul operand and batch is the moving free dim (512 cols per
chunk).  All matmuls run as float32r (FP22) which streams 1 row/cycle on
the PE vs 4 for true fp32.  Per-sample reductions over feature dims are
done on the PE with small mask matmuls that scatter chunk j's reduction
into row j of a [16, 512] stats tile.
"""

import numpy as np
from contextlib import ExitStack

import concourse.bass as bass
import concourse.mybir as mybir
import concourse.tile as tile
from concourse import bacc
from concourse.bass_utils import run_bass_kernel_spmd

F32 = mybir.dt.float32
F32R = mybir.dt.float32r
AF = mybir.ActivationFunctionType
OP = mybir.AluOpType

NF = 128       # state dim
H = 1024       # hidden dim
HK = H // 128  # hidden chunks
C = 32         # control dim
BC = 512       # batch columns per chunk
N_CORES = 8

PENALTY = 100.0


def build_nc(nb: int) -> bass.Bass:
    """One-core program: nb chunks of BC batch columns (feature-major)."""
    Bc = nb * BC
    nc = bacc.Bacc()

    # ---- DRAM I/O ----
    xT_d = nc.dram_tensor("xT", [NF, Bc], F32, kind="ExternalInput")
    w1t_d = nc.dram_tensor("W1T", [NF, H], F32, kind="ExternalInput")
    w2t_d = nc.dram_tensor("W2T", [H, H], F32, kind="ExternalInput")
    w2s_d = nc.dram_tensor("W2S", [H, H], F32, kind="ExternalInput")
    w1s_d = nc.dram_tensor("W1S", [H, NF], F32, kind="ExternalInput")
    aft_d = nc.dram_tensor("AFT", [NF, NF], F32, kind="ExternalInput")
    gm_d = nc.dram_tensor("GM", [NF, C], F32, kind="ExternalInput")
    ktn_d = nc.dram_tensor("KTN", [NF, C], F32, kind="ExternalInput")
    b1c_d = nc.dram_tensor("B1C", [128, HK], F32, kind="ExternalInput")
    b2c_d = nc.dram_tensor("B2C", [128, HK], F32, kind="ExternalInput")
    ema_d = nc.dram_tensor("EMA", [128, 16, 48], F32, kind="ExternalInput")
    bc100_d = nc.dram_tensor("BC100", [1, C], F32, kind="ExternalInput")

    ut_d = nc.dram_tensor("UT", [C, Bc], F32, kind="ExternalOutput")
    ro_d = nc.dram_tensor("RO", [nb, BC], F32, kind="ExternalOutput")
    vo_d = nc.dram_tensor("VO", [nb, BC], F32, kind="ExternalOutput")
    vdo_d = nc.dram_tensor("VDO", [nb, BC], F32, kind="ExternalOutput")

    with ExitStack() as ctx:
        tc = ctx.enter_context(tile.TileContext(nc))
        wp = ctx.enter_context(tc.tile_pool(name="w", bufs=1))
        xp = ctx.enter_context(tc.tile_pool(name="x", bufs=2))
        a1p = ctx.enter_context(tc.tile_pool(name="a1p", bufs=2))
        a2p = ctx.enter_context(tc.tile_pool(name="a2p", bufs=1))
        one = ctx.enter_context(tc.tile_pool(name="one", bufs=1))
        sq2p = ctx.enter_context(tc.tile_pool(name="sq2p", bufs=2))
        gvp = ctx.enter_context(tc.tile_pool(name="gvp", bufs=1))
        keep = ctx.enter_context(tc.tile_pool(name="keep", bufs=nb))
        prodp = ctx.enter_context(tc.tile_pool(name="prodp", bufs=1))
        fin = ctx.enter_context(tc.tile_pool(name="fin", bufs=1))
        psz = ctx.enter_context(tc.tile_pool(name="psz", bufs=3, space="PSUM"))
        pss = ctx.enter_context(tc.tile_pool(name="pss", bufs=3, space="PSUM"))
        psst = ctx.enter_context(tc.tile_pool(name="psst", bufs=1, space="PSUM"))

        def mm(out, lhsT, rhs, start, stop, **kw):
            nc.tensor.matmul(out, lhsT.bitcast(F32R), rhs.bitcast(F32R),
                             start=start, stop=stop, **kw)

        def r32(ap):
            # walrus requires producers of f32r-matmul operands to declare
            # f32r (rounded) output
            return ap.bitcast(F32R)

        # ---- load weights / constants ----
        w1t = wp.tile([NF, H], F32, tag="w1t")
        nc.sync.dma_start(out=r32(w1t[:]), in_=r32(w1t_d[:]))
        w2t = []
        w2s = []
        w1s = []
        for k in range(HK):
            t1 = wp.tile([128, H], F32, tag=f"w2t{k}")
            nc.sync.dma_start(out=r32(t1[:]), in_=r32(w2t_d[k * 128:(k + 1) * 128, :]))
            w2t.append(t1)
            t2 = wp.tile([128, H], F32, tag=f"w2s{k}")
            nc.sync.dma_start(out=r32(t2[:]), in_=r32(w2s_d[k * 128:(k + 1) * 128, :]))
            w2s.append(t2)
            t3 = wp.tile([128, NF], F32, tag=f"w1s{k}")
            nc.sync.dma_start(out=r32(t3[:]), in_=r32(w1s_d[k * 128:(k + 1) * 128, :]))
            w1s.append(t3)
        aft = wp.tile([NF, NF], F32, tag="aft")
        nc.sync.dma_start(out=r32(aft[:]), in_=r32(aft_d[:]))
        gm = wp.tile([NF, C], F32, tag="gm")
        nc.sync.dma_start(out=r32(gm[:]), in_=r32(gm_d[:]))
        ktn = wp.tile([NF, C], F32, tag="ktn")
        nc.sync.dma_start(out=r32(ktn[:]), in_=r32(ktn_d[:]))
        b1c = wp.tile([128, HK], F32, tag="b1c")
        nc.sync.dma_start(out=b1c[:], in_=b1c_d[:])
        b2c = wp.tile([128, HK], F32, tag="b2c")
        nc.sync.dma_start(out=b2c[:], in_=b2c_d[:])
        ema = wp.tile([128, 16, 48], F32, tag="ema")
        nc.sync.dma_start(out=r32(ema[:]), in_=r32(ema_d[:]))
        bc100 = wp.tile([1, C], F32, tag="bc100")
        nc.sync.dma_start(out=r32(bc100[:]), in_=r32(bc100_d[:]))

        # persistent SBUF stats accumulators
        stA_sb = fin.tile([48, BC], F32, tag="stA_sb")  # rows 0:nb=L_f, 32:32+nb=sum a2^2
        stB_sb = fin.tile([48, BC], F32, tag="stB_sb")  # rows 0:nb=<Lg,unom>, 32:32+nb=|Lg|^2
        nc.vector.memset(stA_sb[:], 0.0)
        nc.vector.memset(stB_sb[:], 0.0)

        lgj_tiles = []
        unj_tiles = []

        for j in range(nb):
            xt = xp.tile([NF, BC], F32, tag="xt")
            nc.sync.dma_start(out=r32(xt[:]), in_=r32(xT_d[:, j * BC:(j + 1) * BC]))

            stA = psst.tile([48, BC], F32, tag="stA")

            # ---- layer 1: a1 = tanh(W1 x + b1) ----
            a1t = a1p.tile([128, HK, BC], F32, tag="a1t")
            for m in range(HK):
                z1 = psz.tile([128, BC], F32, tag="zz")
                mm(z1[:], w1t[:, m * 128:(m + 1) * 128], xt[:], True, True)
                nc.scalar.activation(r32(a1t[:, m, :]), z1[:], AF.Tanh,
                                     bias=b1c[:, m:m + 1], scale=1.0)

            # ---- layer 2: a2 = tanh(W2 a1 + b2); sq2; V; md2 = (a2^2-1)a2 ----
            a2t = a2p.tile([128, HK, BC], F32, tag="a2t")
            for m in range(HK):
                z2 = psz.tile([128, BC], F32, tag="zz")
                for k in range(HK):
                    mm(z2[:], w2t[k][:, m * 128:(m + 1) * 128], a1t[:, k, :],
                       k == 0, k == HK - 1)
                nc.scalar.activation(r32(a2t[:, m, :]), z2[:], AF.Tanh,
                                     bias=b2c[:, m:m + 1], scale=1.0)
                sq2 = sq2p.tile([128, BC], F32, tag="sq2")
                nc.vector.tensor_mul(r32(sq2[:]), a2t[:, m, :], a2t[:, m, :])
                # V partial: sum over this hidden chunk -> stats row 8+j
                mm(stA[:], ema[:, 8 + j, :], sq2[:], m == 0, False,
                   skip_group_check=True)
                # md2 = (sq2 - 1) * a2  (= -d2), in place over a2
                nc.vector.scalar_tensor_tensor(
                    out=r32(a2t[:, m, :]), in0=sq2[:], scalar=1.0,
                    in1=a2t[:, m, :], op0=OP.subtract, op1=OP.mult)

            # sq1 = a1^2 (frees a1 afterwards)
            sq1t = one.tile([128, HK, BC], F32, tag="sq1t")
            for m in range(HK):
                nc.vector.tensor_mul(r32(sq1t[:, m, :]), a1t[:, m, :], a1t[:, m, :])

            # ---- t' = W2^T md2 (= -t); d1 = (sq1-1)*t' in place over sq1 ----
            for m in range(HK):
                tp = psz.tile([128, BC], F32, tag="zz")
                for k in range(HK):
                    mm(tp[:], w2s[k][:, m * 128:(m + 1) * 128], a2t[:, k, :],
                       k == 0, k == HK - 1)
                nc.vector.scalar_tensor_tensor(
                    out=r32(sq1t[:, m, :]), in0=sq1t[:, m, :], scalar=1.0,
                    in1=tp[:], op0=OP.subtract, op1=OP.mult)

            # ---- grad_V = W1^T d1 ----
            gv = pss.tile([128, BC], F32, tag="small")
            for m in range(HK):
                mm(gv[:], w1s[m][:], sq1t[:, m, :], m == 0, m == HK - 1)
            gradv = gvp.tile([128, BC], F32, tag="gradv")
            nc.scalar.activation(r32(gradv[:]), gv[:], AF.Copy)

            # ---- f = A_f x; gvf = grad_V . f; L_f -> stats row j ----
            fp = pss.tile([128, BC], F32, tag="small")
            mm(fp[:], aft[:], xt[:], True, True)
            gvf = gvp.tile([128, BC], F32, tag="gvf")
            nc.vector.tensor_mul(r32(gvf[:]), gradv[:], fp[:])
            mm(stA[:], ema[:, j, :], gvf[:], False, True, skip_group_check=True)
            nc.vector.tensor_add(stA_sb[:], stA_sb[:], stA[:])

            # ---- L_g = G^T grad_V; u_nom = -K x ----
            lgp = pss.tile([C, BC], F32, tag="small")
            mm(lgp[:], gm[:], gradv[:], True, True)
            unp = pss.tile([C, BC], F32, tag="small")
            mm(unp[:], ktn[:], xt[:], True, True)
            lgj = keep.tile([C, BC], F32, tag="lgj")
            nc.scalar.activation(lgj[:], lgp[:], AF.Copy)
            unj = keep.tile([C, BC], F32, tag="unj")
            nc.scalar.activation(unj[:], unp[:], AF.Copy)
            lgj_tiles.append(lgj)
            unj_tiles.append(unj)

            # <L_g,u_nom> -> row j ; |L_g|^2 -> row 32+j (mask rows reused
            # from ema: identical on every partition)
            lgu_t = prodp.tile([C, BC], F32, tag="lgu_t")
            nc.vector.tensor_mul(r32(lgu_t[:]), lgj[:], unj[:])
            lg2_t = prodp.tile([C, BC], F32, tag="lg2_t")
            nc.vector.tensor_mul(r32(lg2_t[:]), lgj[:], lgj[:])
            stB = psst.tile([48, BC], F32, tag="stB")
            mm(stB[:], ema[0:C, j, :], lgu_t[:], True, False,
               skip_group_check=True)
            mm(stB[:], ema[0:C, 8 + j, :], lg2_t[:], False, True,
               skip_group_check=True)
            nc.vector.tensor_add(stB_sb[:], stB_sb[:], stB[:])

        # ---- final per-sample scalar math on [nb, BC] tiles ----
        # DVE operands must share partition offsets; DMA the offset-32 rows
        # down to partition 0 first.
        lf = stA_sb[0:nb, :]
        lgu = stB_sb[0:nb, :]
        vsum_t = fin.tile([nb, BC], F32, tag="vsum_t")
        nc.sync.dma_start(out=vsum_t[:], in_=stA_sb[32:32 + nb, :])
        lg2_t2 = fin.tile([nb, BC], F32, tag="lg2_t2")
        nc.sync.dma_start(out=lg2_t2[:], in_=stB_sb[32:32 + nb, :])
        vsum = vsum_t[:]
        lg2 = lg2_t2[:]

        # Scratch tiles are manually reused once their previous value is dead.
        tmp = fin.tile([nb, BC], F32, tag="tmp")
        s = fin.tile([nb, BC], F32, tag="s")
        denom = fin.tile([nb, BC], F32, tag="denom")
        rec = fin.tile([nb, BC], F32, tag="rec")
        rbuf = fin.tile([nb, BC], F32, tag="rbuf")

        # tmp = 0.5*vsum + lf ; s = tmp + lgu
        nc.vector.scalar_tensor_tensor(out=tmp[:], in0=vsum, scalar=0.5,
                                       in1=lf, op0=OP.mult, op1=OP.add)
        nc.vector.tensor_add(s[:], tmp[:], lgu)
        # denom = 100*lg2 + 1 ; rec = 1/denom
        nc.vector.tensor_scalar(out=denom[:], in0=lg2, scalar1=PENALTY,
                                scalar2=1.0, op0=OP.mult, op1=OP.add)
        nc.vector.reciprocal(rec[:], denom[:])
        # tmp := relu(s) ; rbuf = tmp * rec  (= r)
        nc.vector.tensor_scalar_max(tmp[:], s[:], 0.0)
        nc.vector.tensor_mul(rbuf[:], tmp[:], rec[:])
        nc.sync.dma_start(out=ro_d[:], in_=rbuf[:])

        # denom := 0.5 * vsum  (= V output)
        nc.vector.tensor_scalar_mul(denom[:], vsum, 0.5)
        nc.sync.dma_start(out=vo_d[:], in_=denom[:])

        # s := rbuf*lg2 ; tmp := lf+lgu ; rec := -100*s + tmp  (= Vdot)
        nc.vector.tensor_mul(s[:], rbuf[:], lg2)
        nc.vector.tensor_add(tmp[:], lf, lgu)
        nc.vector.scalar_tensor_tensor(out=rec[:], in0=s[:], scalar=-PENALTY,
                                       in1=tmp[:], op0=OP.mult, op1=OP.add)
        nc.sync.dma_start(out=vdo_d[:], in_=rec[:])

        # ---- u = u_nom - (100 r) L_g, chunk by chunk ----
        for j in range(nb):
            # matmul rhs must start at partition 0/32/64 — DMA the r row down.
            # Reuse the (now dead) prodp slots for these final tiles.
            rj = prodp.tile([1, BC], F32, tag="lg2_t")
            nc.sync.dma_start(out=r32(rj[:]), in_=r32(rbuf[j:j + 1, :]))
            prb = pss.tile([C, BC], F32, tag="small")
            mm(prb[:], bc100[:], rj[:], True, True)
            mu = prodp.tile([C, BC], F32, tag="lgu_t")
            nc.vector.tensor_mul(mu[:], lgj_tiles[j][:], prb[:])
            nc.vector.tensor_sub(mu[:], unj_tiles[j][:], mu[:])
            nc.sync.dma_start(out=ut_d[:, j * BC:(j + 1) * BC], in_=mu[:])

    nc.finalize()
    return nc


_NC_CACHE: dict[int, bass.Bass] = {}


def _get_nc(nb: int) -> bass.Bass:
    if nb not in _NC_CACHE:
        _NC_CACHE[nb] = build_nc(nb)
    return _NC_CACHE[nb]


def _asc(a):
    return np.ascontiguousarray(np.asarray(a, dtype=np.float32))


def make_host_inputs(x, W1, b1, W2, b2, A_f, Gmat, K):
    """Host-side layout prep shared by all cores."""
    ema = np.zeros((128, 16, 48), np.float32)
    for i in range(16):
        col = i if i < 8 else 24 + i  # L_f -> row j, V -> row 32+j
        ema[:, i, col] = 1.0

    common = {
        "W1T": _asc(np.asarray(W1).T),
        "W2T": _asc(np.asarray(W2).T),
        "W2S": _asc(W2),
        "W1S": _asc(W1),
        "AFT": _asc(np.asarray(A_f).T),
        "GM": _asc(Gmat),
        "KTN": _asc(-np.asarray(K).T),
        "B1C": _asc(np.asarray(b1).reshape(HK, 128).T),
        "B2C": _asc(np.asarray(b2).reshape(HK, 128).T),
        "EMA": ema,
        "BC100": np.full((1, C), PENALTY, np.float32),
    }
    return common


def kernel(x, W1, b1, W2, b2, A_f, Gmat, K):
    x = _asc(x)
    B = x.shape[0]
    per = B // N_CORES
    nb = per // BC
    nc = _get_nc(nb)
    common = make_host_inputs(x, W1, b1, W2, b2, A_f, Gmat, K)
    xT = _asc(x.T)
    in_maps = [
        {**common, "xT": _asc(xT[:, i * per:(i + 1) * per])}
        for i in range(N_CORES)
    ]
    res = run_bass_kernel_spmd(nc, in_maps, list(range(N_CORES)))
    rs = res.results
    u = np.concatenate([rs[i]["UT"].T for i in range(N_CORES)], axis=0)
    r = np.concatenate([rs[i]["RO"].reshape(per, 1) for i in range(N_CORES)], axis=0)
    V = np.concatenate([rs[i]["VO"].reshape(per) for i in range(N_CORES)], axis=0)
    Vd = np.concatenate([rs[i]["VDO"].reshape(per, 1, 1) for i in range(N_CORES)], axis=0)
    return u, r, V, Vd


# revision 19
# speedup vs baseline: 1.0797x; 1.0797x over previous
"""Trainium2 Bass kernel for the CLF-QP network.

Math (per sample, fp32 reference):
    a1 = tanh(W1 x + b1); a2 = tanh(W2 a1 + b2); V = 0.5||a2||^2
    grad_V = W1^T ((1-a1^2) . (W2^T ((1-a2^2) . a2)))
    f = A_f x; L_f = <grad_V, f>; L_g = Gmat^T grad_V; u_nom = -K x
    s = L_f + V + <L_g, u_nom>; r = relu(s) / (1 + 100||L_g||^2)
    u = u_nom - 100 r L_g; Vdot = L_f + <L_g, u>

Strategy: pure data parallel over 8 NeuronCores (4096 rows each).
On-chip layout is feature-major ([feature, batch]): weights are the
stationary matmul operand, batch the moving free dim (512-col chunks).
Matmul operands are fp16 (weight loads are FWL-fast and fully hidden;
~2x faster pair rate than true fp32), accumulation stays fp32 in PSUM
and all per-sample scalar math + outputs are fp32.  Per-sample
reductions over feature dims run on the PE as small mask matmuls that
scatter chunk j's reduction into dedicated rows of a [48, 512] stats
tile (rows: L_f 0-7, <Lg,u_nom> 8-15, sum a2^2 32-39, |L_g|^2 40-47).
"""

import numpy as np
from contextlib import ExitStack

import concourse.bass as bass
import concourse.mybir as mybir
import concourse.tile as tile
from concourse import bacc
from concourse.bass_utils import run_bass_kernel_spmd

F32 = mybir.dt.float32
F16 = mybir.dt.float16
AF = mybir.ActivationFunctionType
OP = mybir.AluOpType

NF = 128       # state dim
H = 1024       # hidden dim
HK = H // 128  # hidden chunks
C = 32         # control dim
BC = 512       # batch columns per chunk
N_CORES = 8

PENALTY = 100.0


def build_nc(nb: int) -> bass.Bass:
    """One-core program: nb chunks of BC batch columns (feature-major)."""
    Bc = nb * BC
    nc = bacc.Bacc()

    # ---- DRAM I/O (fp16 operands, fp32 biases + outputs) ----
    xT_d = nc.dram_tensor("xT", [NF, Bc], F16, kind="ExternalInput")
    w1t_d = nc.dram_tensor("W1T", [NF, H], F16, kind="ExternalInput")
    w2t_d = nc.dram_tensor("W2T", [H, H], F16, kind="ExternalInput")
    w2s_d = nc.dram_tensor("W2S", [H, H], F16, kind="ExternalInput")
    w1s_d = nc.dram_tensor("W1S", [H, NF], F16, kind="ExternalInput")
    aft_d = nc.dram_tensor("AFT", [NF, NF], F16, kind="ExternalInput")
    gm_d = nc.dram_tensor("GM", [NF, C], F16, kind="ExternalInput")
    ktn_d = nc.dram_tensor("KTN", [NF, C], F16, kind="ExternalInput")
    b1c_d = nc.dram_tensor("B1C", [128, HK], F32, kind="ExternalInput")
    b2c_d = nc.dram_tensor("B2C", [128, HK], F32, kind="ExternalInput")
    ema_d = nc.dram_tensor("EMA", [128, 16, 48], F16, kind="ExternalInput")
    emb2_d = nc.dram_tensor("EMB2", [C, 16, 48], F16, kind="ExternalInput")
    bc100_d = nc.dram_tensor("BC100", [1, C], F16, kind="ExternalInput")

    ut_d = nc.dram_tensor("UT", [C, Bc], F32, kind="ExternalOutput")
    ro_d = nc.dram_tensor("RO", [nb, BC], F32, kind="ExternalOutput")
    vo_d = nc.dram_tensor("VO", [nb, BC], F32, kind="ExternalOutput")
    vdo_d = nc.dram_tensor("VDO", [nb, BC], F32, kind="ExternalOutput")

    with ExitStack() as ctx:
        tc = ctx.enter_context(tile.TileContext(nc))
        wp = ctx.enter_context(tc.tile_pool(name="w", bufs=1))
        xp = ctx.enter_context(tc.tile_pool(name="x", bufs=2))
        a1p = ctx.enter_context(tc.tile_pool(name="a1p", bufs=2))
        a2p = ctx.enter_context(tc.tile_pool(name="a2p", bufs=2))
        one = ctx.enter_context(tc.tile_pool(name="one", bufs=2))
        sq2p = ctx.enter_context(tc.tile_pool(name="sq2p", bufs=3))
        gvp = ctx.enter_context(tc.tile_pool(name="gvp", bufs=2))
        keep = ctx.enter_context(tc.tile_pool(name="keep", bufs=nb))
        prodp = ctx.enter_context(tc.tile_pool(name="prodp", bufs=2))
        fin = ctx.enter_context(tc.tile_pool(name="fin", bufs=1))
        psz = ctx.enter_context(tc.tile_pool(name="psz", bufs=3, space="PSUM"))
        pss = ctx.enter_context(tc.tile_pool(name="pss", bufs=3, space="PSUM"))
        psst = ctx.enter_context(tc.tile_pool(name="psst", bufs=2, space="PSUM"))

        def mm(out, lhsT, rhs, start, stop, **kw):
            nc.tensor.matmul(out, lhsT, rhs, start=start, stop=stop, **kw)

        # ---- load weights / constants (layer-1/2 weights first) ----
        w1t = wp.tile([NF, H], F16, tag="w1t")
        nc.sync.dma_start(out=w1t[:], in_=w1t_d[:])
        b1c = wp.tile([128, HK], F32, tag="b1c")
        nc.sync.dma_start(out=b1c[:], in_=b1c_d[:])
        b2c = wp.tile([128, HK], F32, tag="b2c")
        nc.sync.dma_start(out=b2c[:], in_=b2c_d[:])
        w2t = []
        w2s = []
        w1s = []
        for k in range(HK):
            t1 = wp.tile([128, H], F16, tag=f"w2t{k}")
            nc.sync.dma_start(out=t1[:], in_=w2t_d[k * 128:(k + 1) * 128, :])
            w2t.append(t1)
        aft = wp.tile([NF, NF], F16, tag="aft")
        nc.sync.dma_start(out=aft[:], in_=aft_d[:])
        gm = wp.tile([NF, C], F16, tag="gm")
        nc.sync.dma_start(out=gm[:], in_=gm_d[:])
        ktn = wp.tile([NF, C], F16, tag="ktn")
        nc.sync.dma_start(out=ktn[:], in_=ktn_d[:])
        ema = wp.tile([128, 16, 48], F16, tag="ema")
        nc.sync.dma_start(out=ema[:], in_=ema_d[:])
        emb2 = wp.tile([C, 16, 48], F16, tag="emb2")
        nc.sync.dma_start(out=emb2[:], in_=emb2_d[:])
        bc100 = wp.tile([1, C], F16, tag="bc100")
        nc.sync.dma_start(out=bc100[:], in_=bc100_d[:])
        # W2/W1 (stored) are needed only from the t'/grad_V stages on
        for k in range(HK):
            t2 = wp.tile([128, H], F16, tag=f"w2s{k}")
            nc.sync.dma_start(out=t2[:], in_=w2s_d[k * 128:(k + 1) * 128, :])
            w2s.append(t2)
            t3 = wp.tile([128, NF], F16, tag=f"w1s{k}")
            nc.sync.dma_start(out=t3[:], in_=w1s_d[k * 128:(k + 1) * 128, :])
            w1s.append(t3)

        # persistent SBUF stats accumulator (fp32):
        # rows 0:nb = L_f, 8:8+nb = <Lg,u_nom>, 32:32+nb = sum a2^2,
        # 40:40+nb = |L_g|^2
        st_sb = fin.tile([48, BC], F32, tag="st_sb")
        nc.vector.memset(st_sb[:], 0.0)

        lgj_tiles = []
        unj_tiles = []

        for j in range(nb):
            xt = xp.tile([NF, BC], F16, tag="xt")
            nc.sync.dma_start(out=xt[:], in_=xT_d[:, j * BC:(j + 1) * BC])

            stA = psst.tile([48, BC], F32, tag="stA")

            # ---- layer 1: a1 = tanh(W1 x + b1) ----
            a1t = a1p.tile([128, HK, BC], F16, tag="a1t")
            for m in range(HK):
                z1 = psz.tile([128, BC], F32, tag="zz")
                mm(z1[:], w1t[:, m * 128:(m + 1) * 128], xt[:], True, True)
                nc.scalar.activation(a1t[:, m, :], z1[:], AF.Tanh,
                                     bias=b1c[:, m:m + 1], scale=1.0)

            # ---- layer 2: a2 = tanh(W2 a1 + b2); sq2; V; md2 = (a2^2-1)a2 ----
            a2t = a2p.tile([128, HK, BC], F16, tag="a2t")
            for m in range(HK):
                z2 = psz.tile([128, BC], F32, tag="zz")
                for k in range(HK):
                    mm(z2[:], w2t[k][:, m * 128:(m + 1) * 128], a1t[:, k, :],
                       k == 0, k == HK - 1)
                nc.scalar.activation(a2t[:, m, :], z2[:], AF.Tanh,
                                     bias=b2c[:, m:m + 1], scale=1.0)
                sq2 = sq2p.tile([128, BC], F16, tag="sq2")
                nc.scalar.activation(sq2[:], a2t[:, m, :], AF.Square)
                # V partial: sum over this hidden chunk -> stats row 32+j
                mm(stA[:], ema[:, 8 + j, :], sq2[:], m == 0, False,
                   skip_group_check=True)
                # md2 = (sq2 - 1) * a2  (= -d2), in place over a2
                nc.vector.scalar_tensor_tensor(
                    out=a2t[:, m, :], in0=sq2[:], scalar=1.0,
                    in1=a2t[:, m, :], op0=OP.subtract, op1=OP.mult)

            # sq1 = a1^2 on GpSimd (frees a1 afterwards, keeps DVE light)
            sq1t = one.tile([128, HK, BC], F16, tag="sq1t")
            for m in range(HK):
                nc.gpsimd.tensor_mul(sq1t[:, m, :], a1t[:, m, :], a1t[:, m, :])

            # ---- t' = W2^T md2 (= -t); d1 = (sq1-1)*t' in place over sq1 ----
            for m in range(HK):
                tp = psz.tile([128, BC], F32, tag="zz")
                for k in range(HK):
                    mm(tp[:], w2s[k][:, m * 128:(m + 1) * 128], a2t[:, k, :],
                       k == 0, k == HK - 1)
                nc.vector.scalar_tensor_tensor(
                    out=sq1t[:, m, :], in0=sq1t[:, m, :], scalar=1.0,
                    in1=tp[:], op0=OP.subtract, op1=OP.mult)

            # ---- grad_V = W1^T d1 ----
            gv = pss.tile([128, BC], F32, tag="small")
            for m in range(HK):
                mm(gv[:], w1s[m][:], sq1t[:, m, :], m == 0, m == HK - 1)
            gradv = gvp.tile([128, BC], F16, tag="gradv")
            nc.scalar.activation(gradv[:], gv[:], AF.Copy)

            # ---- f = A_f x; gvf = grad_V . f; L_f -> stats row j ----
            fp = pss.tile([128, BC], F32, tag="small")
            mm(fp[:], aft[:], xt[:], True, True)
            gvf = gvp.tile([128, BC], F16, tag="gvf")
            nc.vector.tensor_mul(gvf[:], gradv[:], fp[:])
            mm(stA[:], ema[:, j, :], gvf[:], False, False,
               skip_group_check=True)

            # ---- L_g = G^T grad_V; u_nom = -K x ----
            lgp = pss.tile([C, BC], F32, tag="small")
            mm(lgp[:], gm[:], gradv[:], True, True)
            unp = pss.tile([C, BC], F32, tag="small")
            mm(unp[:], ktn[:], xt[:], True, True)
            lgj = keep.tile([C, BC], F16, tag="lgj")
            nc.scalar.activation(lgj[:], lgp[:], AF.Copy)
            unj = keep.tile([C, BC], F16, tag="unj")
            nc.scalar.activation(unj[:], unp[:], AF.Copy)
            lgj_tiles.append(lgj)
            unj_tiles.append(unj)

            # <L_g,u_nom> -> stats row 8+j ; |L_g|^2 -> stats row 40+j
            lgu_t = prodp.tile([C, BC], F16, tag="lgu_t")
            nc.vector.tensor_mul(lgu_t[:], lgj[:], unj[:])
            lg2_t = prodp.tile([C, BC], F16, tag="lg2_t")
            nc.vector.tensor_mul(lg2_t[:], lgj[:], lgj[:])
            mm(stA[:], emb2[:, j, :], lgu_t[:], False, False,
               skip_group_check=True)
            mm(stA[:], emb2[:, 8 + j, :], lg2_t[:], False, True,
               skip_group_check=True)
            nc.vector.tensor_add(st_sb[:], st_sb[:], stA[:])

        # ---- final per-sample scalar math on [nb, BC] fp32 tiles ----
        # DVE operands must share partition offsets; DMA the offset rows
        # down to partition 0 first.
        lf = st_sb[0:nb, :]
        lgu_t3 = fin.tile([nb, BC], F32, tag="lgu_t3")
        nc.sync.dma_start(out=lgu_t3[:], in_=st_sb[8:8 + nb, :])
        lgu = lgu_t3[:]
        vsum_t = fin.tile([nb, BC], F32, tag="vsum_t")
        nc.sync.dma_start(out=vsum_t[:], in_=st_sb[32:32 + nb, :])
        lg2_t2 = fin.tile([nb, BC], F32, tag="lg2_t2")
        nc.sync.dma_start(out=lg2_t2[:], in_=st_sb[40:40 + nb, :])
        vsum = vsum_t[:]
        lg2 = lg2_t2[:]

        tmp = fin.tile([nb, BC], F32, tag="tmp")
        s = fin.tile([nb, BC], F32, tag="s")
        denom = fin.tile([nb, BC], F32, tag="denom")
        rbuf = fin.tile([nb, BC], F32, tag="rbuf")

        # tmp = 0.5*vsum + lf ; s = tmp + lgu
        nc.vector.scalar_tensor_tensor(out=tmp[:], in0=vsum, scalar=0.5,
                                       in1=lf, op0=OP.mult, op1=OP.add)
        nc.vector.tensor_add(s[:], tmp[:], lgu)
        # denom = 100*lg2 + 1 ; denom = 1/denom (in place)
        nc.vector.tensor_scalar(out=denom[:], in0=lg2, scalar1=PENALTY,
                                scalar2=1.0, op0=OP.mult, op1=OP.add)
        nc.vector.reciprocal(denom[:], denom[:])
        # tmp := relu(s) ; rbuf = tmp * (1/denom)  (= r)
        nc.vector.tensor_scalar_max(tmp[:], s[:], 0.0)
        nc.vector.tensor_mul(rbuf[:], tmp[:], denom[:])
        nc.sync.dma_start(out=ro_d[:], in_=rbuf[:])

        # vsum_t := 0.5 * vsum  (= V output)
        nc.vector.tensor_scalar_mul(vsum_t[:], vsum, 0.5)
        nc.sync.dma_start(out=vo_d[:], in_=vsum_t[:])

        # s := rbuf*lg2 ; tmp := lf+lgu ; denom := -100*s + tmp  (= Vdot)
        nc.vector.tensor_mul(s[:], rbuf[:], lg2)
        nc.vector.tensor_add(tmp[:], lf, lgu)
        nc.vector.scalar_tensor_tensor(out=denom[:], in0=s[:], scalar=-PENALTY,
                                       in1=tmp[:], op0=OP.mult, op1=OP.add)
        nc.sync.dma_start(out=vdo_d[:], in_=denom[:])

        # fp16 copy of r for the broadcast matmuls
        rh = fin.tile([nb, BC], F16, tag="rh")
        nc.scalar.activation(rh[:], rbuf[:], AF.Copy)

        # ---- u = u_nom - (100 r) L_g, chunk by chunk ----
        for j in range(nb):
            # matmul rhs must start at partition 0/32/64 — DMA the r row down
            rj = prodp.tile([1, BC], F16, tag="lg2_t")
            nc.sync.dma_start(out=rj[:], in_=rh[j:j + 1, :])
            prb = pss.tile([C, BC], F32, tag="small")
            mm(prb[:], bc100[:], rj[:], True, True)
            mu = prodp.tile([C, BC], F32, tag="mu")
            nc.vector.tensor_mul(mu[:], lgj_tiles[j][:], prb[:])
            nc.vector.tensor_sub(mu[:], unj_tiles[j][:], mu[:])
            nc.sync.dma_start(out=ut_d[:, j * BC:(j + 1) * BC], in_=mu[:])

    nc.finalize()
    return nc


_NC_CACHE: dict[int, bass.Bass] = {}


def _get_nc(nb: int) -> bass.Bass:
    if nb not in _NC_CACHE:
        _NC_CACHE[nb] = build_nc(nb)
    return _NC_CACHE[nb]


def _asc(a, dt=np.float16):
    return np.ascontiguousarray(np.asarray(a, dtype=np.float32).astype(dt))


def make_host_inputs(x, W1, b1, W2, b2, A_f, Gmat, K):
    """Host-side layout prep shared by all cores (fp16 operands)."""
    ema = np.zeros((128, 16, 48), np.float16)
    for i in range(16):
        col = i if i < 8 else 24 + i  # L_f -> row j, V -> row 32+j
        ema[:, i, col] = 1.0
    emb2 = np.zeros((C, 16, 48), np.float16)
    for j in range(8):
        emb2[:, j, 8 + j] = 1.0       # <Lg,u_nom> -> row 8+j
        emb2[:, 8 + j, 40 + j] = 1.0  # |Lg|^2 -> row 40+j
    W1 = np.asarray(W1, dtype=np.float32)
    W2 = np.asarray(W2, dtype=np.float32)
    common = {
        "W1T": _asc(W1.T),
        "W2T": _asc(W2.T),
        "W2S": _asc(W2),
        "W1S": _asc(W1),
        "AFT": _asc(np.asarray(A_f, dtype=np.float32).T),
        "GM": _asc(Gmat),
        "KTN": _asc(-np.asarray(K, dtype=np.float32).T),
        "B1C": _asc(np.asarray(b1, dtype=np.float32).reshape(HK, 128).T, np.float32),
        "B2C": _asc(np.asarray(b2, dtype=np.float32).reshape(HK, 128).T, np.float32),
        "EMA": ema,
        "EMB2": emb2,
        "BC100": np.full((1, C), PENALTY, np.float16),
    }
    return common


def kernel(x, W1, b1, W2, b2, A_f, Gmat, K):
    x = np.ascontiguousarray(np.asarray(x, dtype=np.float32))
    B = x.shape[0]
    per = B // N_CORES
    nb = per // BC
    nc = _get_nc(nb)
    common = make_host_inputs(x, W1, b1, W2, b2, A_f, Gmat, K)
    xT = np.ascontiguousarray(x.T.astype(np.float16))
    in_maps = [
        {**common, "xT": np.ascontiguousarray(xT[:, i * per:(i + 1) * per])}
        for i in range(N_CORES)
    ]
    res = run_bass_kernel_spmd(nc, in_maps, list(range(N_CORES)))
    rs = res.results
    u = np.concatenate([rs[i]["UT"].T for i in range(N_CORES)], axis=0)
    r = np.concatenate([rs[i]["RO"].reshape(per, 1) for i in range(N_CORES)], axis=0)
    V = np.concatenate([rs[i]["VO"].reshape(per) for i in range(N_CORES)], axis=0)
    Vd = np.concatenate([rs[i]["VDO"].reshape(per, 1, 1) for i in range(N_CORES)], axis=0)
    return u, r, V, Vd


# revision 22
# speedup vs baseline: 1.1292x; 1.0458x over previous
"""Trainium2 Bass kernel for the CLF-QP network.

Math (per sample, fp32 reference):
    a1 = tanh(W1 x + b1); a2 = tanh(W2 a1 + b2); V = 0.5||a2||^2
    grad_V = W1^T ((1-a1^2) . (W2^T ((1-a2^2) . a2)))
    f = A_f x; L_f = <grad_V, f>; L_g = Gmat^T grad_V; u_nom = -K x
    s = L_f + V + <L_g, u_nom>; r = relu(s) / (1 + 100||L_g||^2)
    u = u_nom - 100 r L_g; Vdot = L_f + <L_g, u>

Strategy: pure data parallel over 8 NeuronCores (4096 rows each).
On-chip layout is feature-major ([feature, batch]): weights are the
stationary matmul operand, batch the moving free dim (512-col chunks).
Matmul operands are fp16 (weight loads are FWL-fast and fully hidden;
~2x faster pair rate than true fp32), accumulation stays fp32 in PSUM
and all per-sample scalar math + outputs are fp32.  Per-sample
reductions over feature dims run on the PE as small mask matmuls that
scatter chunk j's reduction into dedicated rows of [48, 512] stats
tiles (stA: L_f 0-7, sum a2^2 32-39; stB: <Lg,u_nom> 0-7, |L_g|^2 32-39).
"""

import numpy as np
from contextlib import ExitStack

import concourse.bass as bass
import concourse.mybir as mybir
import concourse.tile as tile
from concourse import bacc
from concourse.bass_utils import run_bass_kernel_spmd

F32 = mybir.dt.float32
F16 = mybir.dt.float16
AF = mybir.ActivationFunctionType
OP = mybir.AluOpType

NF = 128       # state dim
H = 1024       # hidden dim
HK = H // 128  # hidden chunks
C = 32         # control dim
BC = 512       # batch columns per chunk
N_CORES = 8

PENALTY = 100.0


def build_nc(nb: int) -> bass.Bass:
    """One-core program: nb chunks of BC batch columns (feature-major)."""
    Bc = nb * BC
    nc = bacc.Bacc()

    # ---- DRAM I/O (fp16 operands, fp32 biases + outputs) ----
    xT_d = nc.dram_tensor("xT", [NF, Bc], F16, kind="ExternalInput")
    w1t_d = nc.dram_tensor("W1T", [NF, H], F16, kind="ExternalInput")
    w2t_d = nc.dram_tensor("W2T", [H, H], F16, kind="ExternalInput")
    w2s_d = nc.dram_tensor("W2S", [H, H], F16, kind="ExternalInput")
    w1s_d = nc.dram_tensor("W1S", [H, NF], F16, kind="ExternalInput")
    aft_d = nc.dram_tensor("AFT", [NF, NF], F16, kind="ExternalInput")
    gm_d = nc.dram_tensor("GM", [NF, C], F16, kind="ExternalInput")
    ktn_d = nc.dram_tensor("KTN", [NF, C], F16, kind="ExternalInput")
    b1c_d = nc.dram_tensor("B1C", [128, HK], F32, kind="ExternalInput")
    b2c_d = nc.dram_tensor("B2C", [128, HK], F32, kind="ExternalInput")
    ema_d = nc.dram_tensor("EMA", [128, 16, 48], F16, kind="ExternalInput")
    bc100_d = nc.dram_tensor("BC100", [1, C], F16, kind="ExternalInput")

    ut_d = nc.dram_tensor("UT", [C, Bc], F32, kind="ExternalOutput")
    ro_d = nc.dram_tensor("RO", [nb, BC], F32, kind="ExternalOutput")
    vo_d = nc.dram_tensor("VO", [nb, BC], F32, kind="ExternalOutput")
    vdo_d = nc.dram_tensor("VDO", [nb, BC], F32, kind="ExternalOutput")

    with ExitStack() as ctx:
        tc = ctx.enter_context(tile.TileContext(nc))
        wp = ctx.enter_context(tc.tile_pool(name="w", bufs=1))
        xp = ctx.enter_context(tc.tile_pool(name="x", bufs=2))
        a1p = ctx.enter_context(tc.tile_pool(name="a1p", bufs=2))
        a2p = ctx.enter_context(tc.tile_pool(name="a2p", bufs=2))
        one = ctx.enter_context(tc.tile_pool(name="one", bufs=2))
        sq2p = ctx.enter_context(tc.tile_pool(name="sq2p", bufs=3))
        gvp = ctx.enter_context(tc.tile_pool(name="gvp", bufs=2))
        keep = ctx.enter_context(tc.tile_pool(name="keep", bufs=nb))
        prodp = ctx.enter_context(tc.tile_pool(name="prodp", bufs=2))
        fin = ctx.enter_context(tc.tile_pool(name="fin", bufs=1))
        psz = ctx.enter_context(tc.tile_pool(name="psz", bufs=3, space="PSUM"))
        pss = ctx.enter_context(tc.tile_pool(name="pss", bufs=3, space="PSUM"))
        psst = ctx.enter_context(tc.tile_pool(name="psst", bufs=1, space="PSUM"))

        def mm(out, lhsT, rhs, start, stop, **kw):
            nc.tensor.matmul(out, lhsT, rhs, start=start, stop=stop, **kw)

        # ---- load weights / constants (layer-1/2 weights first) ----
        w1t = wp.tile([NF, H], F16, tag="w1t")
        nc.sync.dma_start(out=w1t[:], in_=w1t_d[:])
        b1c = wp.tile([128, HK], F32, tag="b1c")
        nc.sync.dma_start(out=b1c[:], in_=b1c_d[:])
        b2c = wp.tile([128, HK], F32, tag="b2c")
        nc.sync.dma_start(out=b2c[:], in_=b2c_d[:])
        w2t = []
        w2s = []
        w1s = []
        for k in range(HK):
            t1 = wp.tile([128, H], F16, tag=f"w2t{k}")
            nc.sync.dma_start(out=t1[:], in_=w2t_d[k * 128:(k + 1) * 128, :])
            w2t.append(t1)
        aft = wp.tile([NF, NF], F16, tag="aft")
        nc.sync.dma_start(out=aft[:], in_=aft_d[:])
        gm = wp.tile([NF, C], F16, tag="gm")
        nc.sync.dma_start(out=gm[:], in_=gm_d[:])
        ktn = wp.tile([NF, C], F16, tag="ktn")
        nc.sync.dma_start(out=ktn[:], in_=ktn_d[:])
        ema = wp.tile([128, 16, 48], F16, tag="ema")
        nc.sync.dma_start(out=ema[:], in_=ema_d[:])
        bc100 = wp.tile([1, C], F16, tag="bc100")
        nc.sync.dma_start(out=bc100[:], in_=bc100_d[:])

        # persistent SBUF stats accumulators (fp32):
        # stA: rows 0:nb = L_f, 32:32+nb = sum a2^2
        # stB: rows 0:nb = <Lg,u_nom>, 32:32+nb = |L_g|^2
        stA_sb = fin.tile([48, BC], F32, tag="stA_sb")
        nc.vector.memset(stA_sb[:], 0.0)
        stB_sb = fin.tile([48, BC], F32, tag="stB_sb")
        nc.vector.memset(stB_sb[:], 0.0)

        lgj_tiles = []
        unj_tiles = []

        def emit_final(j0, j1):
            """Per-sample QP math + u output for chunks [j0, j1).

            All stats rows are DMA'd down to partition 0 first (DVE
            operands must share partition offsets)."""
            cnt = j1 - j0
            lf_t = fin.tile([cnt, BC], F32, tag="lf_t")
            nc.sync.dma_start(out=lf_t[:], in_=stA_sb[j0:j1, :])
            lgu_t3 = fin.tile([cnt, BC], F32, tag="lgu_t3")
            nc.sync.dma_start(out=lgu_t3[:], in_=stB_sb[j0:j1, :])
            vsum_t = fin.tile([cnt, BC], F32, tag="vsum_t")
            nc.sync.dma_start(out=vsum_t[:], in_=stA_sb[32 + j0:32 + j1, :])
            lg2_t2 = fin.tile([cnt, BC], F32, tag="lg2_t2")
            nc.sync.dma_start(out=lg2_t2[:], in_=stB_sb[32 + j0:32 + j1, :])
            lf = lf_t[:]
            lgu = lgu_t3[:]
            vsum = vsum_t[:]
            lg2 = lg2_t2[:]

            tmp = fin.tile([cnt, BC], F32, tag="tmp")
            s = fin.tile([cnt, BC], F32, tag="s")
            denom = fin.tile([cnt, BC], F32, tag="denom")
            rec = fin.tile([cnt, BC], F32, tag="rec")
            rbuf = fin.tile([cnt, BC], F32, tag="rbuf")

            # tmp = 0.5*vsum + lf ; s = tmp + lgu
            nc.vector.scalar_tensor_tensor(out=tmp[:], in0=vsum, scalar=0.5,
                                           in1=lf, op0=OP.mult, op1=OP.add)
            nc.vector.tensor_add(s[:], tmp[:], lgu)
            # denom = 100*lg2 + 1 ; rec = 1/denom
            nc.vector.tensor_scalar(out=denom[:], in0=lg2, scalar1=PENALTY,
                                    scalar2=1.0, op0=OP.mult, op1=OP.add)
            nc.vector.reciprocal(rec[:], denom[:])
            # tmp := relu(s) ; rbuf = tmp * (1/denom)  (= r)
            nc.vector.tensor_scalar_max(tmp[:], s[:], 0.0)
            nc.vector.tensor_mul(rbuf[:], tmp[:], rec[:])
            nc.sync.dma_start(out=ro_d[j0:j1, :], in_=rbuf[:])

            # vsum_t := 0.5 * vsum  (= V output)
            nc.vector.tensor_scalar_mul(vsum_t[:], vsum, 0.5)
            nc.sync.dma_start(out=vo_d[j0:j1, :], in_=vsum_t[:])

            # s := rbuf*lg2 ; tmp := lf+lgu ; denom := -100*s + tmp (= Vdot)
            nc.vector.tensor_mul(s[:], rbuf[:], lg2)
            nc.vector.tensor_add(tmp[:], lf, lgu)
            nc.vector.scalar_tensor_tensor(out=denom[:], in0=s[:],
                                           scalar=-PENALTY, in1=tmp[:],
                                           op0=OP.mult, op1=OP.add)
            nc.sync.dma_start(out=vdo_d[j0:j1, :], in_=denom[:])

            # fp16 copy of r for the broadcast matmuls
            rh = fin.tile([cnt, BC], F16, tag="rh")
            nc.scalar.activation(rh[:], rbuf[:], AF.Copy)

            # u = u_nom - (100 r) L_g, chunk by chunk
            for j in range(j0, j1):
                rj = prodp.tile([1, BC], F16, tag="lg2_t")
                nc.sync.dma_start(out=rj[:], in_=rh[j - j0:j - j0 + 1, :])
                prb = pss.tile([C, BC], F32, tag="small")
                mm(prb[:], bc100[:], rj[:], True, True)
                mu = prodp.tile([C, BC], F32, tag="mu")
                nc.vector.tensor_mul(mu[:], lgj_tiles[j][:], prb[:])
                nc.vector.tensor_sub(mu[:], unj_tiles[j][:], mu[:])
                nc.sync.dma_start(out=ut_d[:, j * BC:(j + 1) * BC], in_=mu[:])

        for j in range(nb):
            xt = xp.tile([NF, BC], F16, tag="xt")
            nc.sync.dma_start(out=xt[:], in_=xT_d[:, j * BC:(j + 1) * BC])

            stA = psst.tile([48, BC], F32, tag="stA")

            # ---- layer 1: a1 = tanh(W1 x + b1) ----
            a1t = a1p.tile([128, HK, BC], F16, tag="a1t")
            for m in range(HK):
                z1 = psz.tile([128, BC], F32, tag="zz")
                mm(z1[:], w1t[:, m * 128:(m + 1) * 128], xt[:], True, True)
                nc.scalar.activation(a1t[:, m, :], z1[:], AF.Tanh,
                                     bias=b1c[:, m:m + 1], scale=1.0)

            # ---- layer 2: a2 = tanh(W2 a1 + b2); sq2; V; md2 = (a2^2-1)a2 ----
            a2t = a2p.tile([128, HK, BC], F16, tag="a2t")
            for m in range(HK):
                z2 = psz.tile([128, BC], F32, tag="zz")
                for k in range(HK):
                    mm(z2[:], w2t[k][:, m * 128:(m + 1) * 128], a1t[:, k, :],
                       k == 0, k == HK - 1)
                nc.scalar.activation(a2t[:, m, :], z2[:], AF.Tanh,
                                     bias=b2c[:, m:m + 1], scale=1.0)
                sq2 = sq2p.tile([128, BC], F16, tag="sq2")
                nc.scalar.activation(sq2[:], a2t[:, m, :], AF.Square)
                # V partial: sum over this hidden chunk -> stats row 32+j
                mm(stA[:], ema[:, 8 + j, :], sq2[:], m == 0, False,
                   skip_group_check=True)
                # md2 = (sq2 - 1) * a2  (= -d2), in place over a2
                nc.vector.scalar_tensor_tensor(
                    out=a2t[:, m, :], in0=sq2[:], scalar=1.0,
                    in1=a2t[:, m, :], op0=OP.subtract, op1=OP.mult)

            if j == 0:
                # W2/W1 (stored) are needed only from the t'/grad_V stages —
                # issue their DMAs after layer-2 so they don't steal HBM
                # bandwidth from the critical-path W2T load at kernel start.
                for k in range(HK):
                    t2 = wp.tile([128, H], F16, tag=f"w2s{k}")
                    nc.sync.dma_start(out=t2[:],
                                      in_=w2s_d[k * 128:(k + 1) * 128, :])
                    w2s.append(t2)
                    t3 = wp.tile([128, NF], F16, tag=f"w1s{k}")
                    nc.sync.dma_start(out=t3[:],
                                      in_=w1s_d[k * 128:(k + 1) * 128, :])
                    w1s.append(t3)

            # sq1 = a1^2 (frees a1 afterwards)
            sq1t = one.tile([128, HK, BC], F16, tag="sq1t")
            for m in range(HK):
                nc.vector.tensor_mul(sq1t[:, m, :], a1t[:, m, :], a1t[:, m, :])

            # ---- t' = W2^T md2 (= -t); d1 = (sq1-1)*t' in place over sq1 ----
            for m in range(HK):
                tp = psz.tile([128, BC], F32, tag="zz")
                for k in range(HK):
                    mm(tp[:], w2s[k][:, m * 128:(m + 1) * 128], a2t[:, k, :],
                       k == 0, k == HK - 1)
                nc.vector.scalar_tensor_tensor(
                    out=sq1t[:, m, :], in0=sq1t[:, m, :], scalar=1.0,
                    in1=tp[:], op0=OP.subtract, op1=OP.mult)

            # ---- grad_V = W1^T d1 ----
            gv = pss.tile([128, BC], F32, tag="small")
            for m in range(HK):
                mm(gv[:], w1s[m][:], sq1t[:, m, :], m == 0, m == HK - 1)
            gradv = gvp.tile([128, BC], F16, tag="gradv")
            nc.scalar.activation(gradv[:], gv[:], AF.Copy)

            # ---- f = A_f x; gvf = grad_V . f; L_f -> stats row j ----
            fp = pss.tile([128, BC], F32, tag="small")
            mm(fp[:], aft[:], xt[:], True, True)
            gvf = gvp.tile([128, BC], F16, tag="gvf")
            nc.vector.tensor_mul(gvf[:], gradv[:], fp[:])
            mm(stA[:], ema[:, j, :], gvf[:], False, True,
               skip_group_check=True)
            nc.vector.tensor_add(stA_sb[:], stA_sb[:], stA[:])

            # ---- L_g = G^T grad_V; u_nom = -K x ----
            lgp = pss.tile([C, BC], F32, tag="small")
            mm(lgp[:], gm[:], gradv[:], True, True)
            unp = pss.tile([C, BC], F32, tag="small")
            mm(unp[:], ktn[:], xt[:], True, True)
            lgj = keep.tile([C, BC], F16, tag="lgj")
            nc.scalar.activation(lgj[:], lgp[:], AF.Copy)
            unj = keep.tile([C, BC], F16, tag="unj")
            nc.scalar.activation(unj[:], unp[:], AF.Copy)
            lgj_tiles.append(lgj)
            unj_tiles.append(unj)

            # <L_g,u_nom> -> stB row j ; |L_g|^2 -> stB row 32+j (mask
            # columns reused from ema; identical on every partition)
            lgu_t = prodp.tile([C, BC], F16, tag="lgu_t")
            nc.vector.tensor_mul(lgu_t[:], lgj[:], unj[:])
            lg2_t = prodp.tile([C, BC], F16, tag="lg2_t")
            nc.vector.tensor_mul(lg2_t[:], lgj[:], lgj[:])
            stB = psst.tile([48, BC], F32, tag="stB")
            mm(stB[:], ema[0:C, j, :], lgu_t[:], True, False,
               skip_group_check=True)
            mm(stB[:], ema[0:C, 8 + j, :], lg2_t[:], False, True,
               skip_group_check=True)
            nc.vector.tensor_add(stB_sb[:], stB_sb[:], stB[:])

            if nb >= 2 and j == nb - 2:
                # chunks 0..nb-2 are final now — their QP math and u output
                # overlap with the last chunk's matmuls
                emit_final(0, nb - 1)

        emit_final(nb - 1, nb)

    nc.finalize()
    return nc


_NC_CACHE: dict[int, bass.Bass] = {}


def _get_nc(nb: int) -> bass.Bass:
    if nb not in _NC_CACHE:
        _NC_CACHE[nb] = build_nc(nb)
    return _NC_CACHE[nb]


def _asc(a, dt=np.float16):
    return np.ascontiguousarray(np.asarray(a, dtype=np.float32).astype(dt))


def make_host_inputs(x, W1, b1, W2, b2, A_f, Gmat, K):
    """Host-side layout prep shared by all cores (fp16 operands)."""
    ema = np.zeros((128, 16, 48), np.float16)
    for i in range(16):
        col = i if i < 8 else 24 + i  # L_f -> row j, V -> row 32+j
        ema[:, i, col] = 1.0
    W1 = np.asarray(W1, dtype=np.float32)
    W2 = np.asarray(W2, dtype=np.float32)
    common = {
        "W1T": _asc(W1.T),
        "W2T": _asc(W2.T),
        "W2S": _asc(W2),
        "W1S": _asc(W1),
        "AFT": _asc(np.asarray(A_f, dtype=np.float32).T),
        "GM": _asc(Gmat),
        "KTN": _asc(-np.asarray(K, dtype=np.float32).T),
        "B1C": _asc(np.asarray(b1, dtype=np.float32).reshape(HK, 128).T, np.float32),
        "B2C": _asc(np.asarray(b2, dtype=np.float32).reshape(HK, 128).T, np.float32),
        "EMA": ema,
        "BC100": np.full((1, C), PENALTY, np.float16),
    }
    return common


def kernel(x, W1, b1, W2, b2, A_f, Gmat, K):
    x = np.ascontiguousarray(np.asarray(x, dtype=np.float32))
    B = x.shape[0]
    per = B // N_CORES
    nb = per // BC
    nc = _get_nc(nb)
    common = make_host_inputs(x, W1, b1, W2, b2, A_f, Gmat, K)
    xT = np.ascontiguousarray(x.T.astype(np.float16))
    in_maps = [
        {**common, "xT": np.ascontiguousarray(xT[:, i * per:(i + 1) * per])}
        for i in range(N_CORES)
    ]
    res = run_bass_kernel_spmd(nc, in_maps, list(range(N_CORES)))
    rs = res.results
    u = np.concatenate([rs[i]["UT"].T for i in range(N_CORES)], axis=0)
    r = np.concatenate([rs[i]["RO"].reshape(per, 1) for i in range(N_CORES)], axis=0)
    V = np.concatenate([rs[i]["VO"].reshape(per) for i in range(N_CORES)], axis=0)
    Vd = np.concatenate([rs[i]["VDO"].reshape(per, 1, 1) for i in range(N_CORES)], axis=0)
    return u, r, V, Vd
